# revision 5
# baseline (speedup 1.0000x reference)
"""Fused FBP (ramp-filter + backprojection + flip + resize + crop) Trainium2 kernel.

The whole reference pipeline is linear in the input sinogram, so it folds into a
single constant matrix T of shape (A*DET, W*W) = (20736, 9216):

    out[n, p] = sum_k x_flat[n, k] * T[k, p]

The kernel is then one PSUM-accumulated matmul per core, with T's output-pixel
axis sharded across the 8 NeuronCores (1152 columns each). T is built on host
once (numpy) and streamed from HBM in bf16; x is replicated in bf16.
"""

import numpy as np
import ml_dtypes

N_ANGLES = 216
DET = 96
WIDTH = 96
UPSAMPLE = 1.8
PAD = 256

SLICES = 96                    # 2*1*48 sinogram slices
K = N_ANGLES * DET             # 20736 contraction length
P_TOTAL = WIDTH * WIDTH        # 9216 output pixels per slice
NCORES = 8
PSH = P_TOTAL // NCORES        # 1152 output pixels per core
KC = K // 128                  # 162 k-chunks of 128
G = 6                          # k-chunks per DMA group (162 = 27 * 6)

_cache = {}


def _build_T():
    """T[(a,d), (Y,X)] such that out = x_flat @ T reproduces the reference."""
    # --- ramp filter as a circular-convolution matrix (filt = sino @ F) ---
    n = np.concatenate((np.arange(1, PAD // 2 + 1, 2), np.arange(PAD // 2 - 1, 0, -2)))
    f = np.zeros(PAD)
    f[0] = 0.25
    f[1::2] = -1.0 / (np.pi * n) ** 2
    full = 2.0 * np.real(np.fft.fft(f))
    ramp_bins = full[: PAD // 2 + 1].astype(np.float32).astype(np.float64)
    kern = np.fft.irfft(ramp_bins, n=PAD)
    s = np.pi / (2.0 * N_ANGLES)
    jj = np.arange(DET)[:, None]
    ii = np.arange(DET)[None, :]
    F = (s * kern[(ii - jj) % PAD]).astype(np.float32)      # (DET j, DET d)

    # --- backprojection weights as hat functions: W[a,d,p] = relu(1-|d-uc|)*inb ---
    angles = np.linspace(0.0, np.pi, N_ANGLES).astype(np.float32).astype(np.float64)
    grid = np.arange(WIDTH) - (WIDTH - 1) / 2.0
    ys, xs = np.meshgrid(grid, grid, indexing="ij")
    t = xs[None] * np.cos(angles)[:, None, None] + ys[None] * np.sin(angles)[:, None, None]
    u = t + (DET - 1) / 2.0                                  # (A, W, W)
    inb = ((u >= 0.0) & (u <= DET - 1)).astype(np.float32)
    uc = np.clip(u, 0.0, DET - 1).astype(np.float32)
    uc_flat = (uc.reshape(N_ANGLES, P_TOTAL) * inb.reshape(N_ANGLES, P_TOTAL))
    inb_flat = inb.reshape(N_ANGLES, P_TOTAL)
    d = np.arange(DET, dtype=np.float32)
    # W built densely per angle, folded with the filter immediately: T1 = F @ W_a
    T1 = np.empty((N_ANGLES, DET, P_TOTAL), dtype=np.float32)
    for a in range(N_ANGLES):
        Wa = np.maximum(0.0, 1.0 - np.abs(d[:, None] - uc_flat[a][None, :])) * inb_flat[a][None, :]
        T1[a] = F.T @ Wa

    # --- flip both spatial dims ---
    T1 = T1.reshape(N_ANGLES, DET, WIDTH, WIDTH)[:, :, ::-1, ::-1]

    # --- upsample(1.8, linear, align_corners=False) + center-crop as one matrix ---
    up = int(WIDTH * UPSAMPLE)
    crop = (up - WIDTH) // 2
    coords = (np.arange(up) + 0.5) * (WIDTH / up) - 0.5
    coords = np.clip(coords, 0.0, WIDTH - 1)
    i0 = np.floor(coords).astype(np.int64)
    i1 = np.minimum(i0 + 1, WIDTH - 1)
    w = (coords - i0).astype(np.float32)
    C = np.zeros((WIDTH, up), dtype=np.float32)
    np.add.at(C, (i0, np.arange(up)), 1.0 - w)
    np.add.at(C, (i1, np.arange(up)), w)
    C = np.ascontiguousarray(C[:, crop : crop + WIDTH])      # (y in, Y out)

    # fold C along y then x: T[(a,j),(Y,X)] = sum_{y,x} T1[a,j,y,x] C[y,Y] C[x,X]
    T2 = np.tensordot(T1, C, axes=([2], [0]))                # (A, DET, X, Y)
    T2 = np.tensordot(T2, C, axes=([2], [0]))                # (A, DET, Y, X)
    return T2.reshape(K, P_TOTAL)


def _build_bass():
    import concourse.bass as bass
    import concourse.mybir as mybir

    B = 4                      # tt ring depth
    NG = KC // G               # 27 DMA groups

    nc = bass.Bass()
    xt = nc.declare_dram_parameter("xt", [128, KC * SLICES], mybir.dt.bfloat16, isOutput=False)
    tsh = nc.declare_dram_parameter("tsh", [KC, 128, PSH], mybir.dt.bfloat16, isOutput=False)
    out = nc.declare_dram_parameter("out", [SLICES, PSH], mybir.dt.float32, isOutput=True)

    from contextlib import ExitStack

    with ExitStack() as stack:
        xt_sb = stack.enter_context(nc.sbuf_tensor([128, KC * SLICES], mybir.dt.bfloat16))
        tt = stack.enter_context(nc.sbuf_tensor([128, B, G, PSH], mybir.dt.bfloat16))
        psum = stack.enter_context(nc.psum_tensor([SLICES, PSH], mybir.dt.float32))
        o_sb = stack.enter_context(nc.sbuf_tensor([SLICES, PSH], mybir.dt.float32))
        xt_sem = stack.enter_context(nc.semaphore("xt_sem"))
        # one DMA in flight per semaphore: SWDGE completions of concurrent DMAs
        # interleave per-engine, so a shared counter cannot order them
        dma_sems = [stack.enter_context(nc.semaphore(f"dma_sem{b}")) for b in range(B)]
        pe_sem = stack.enter_context(nc.semaphore("pe_sem"))
        copy_sem = stack.enter_context(nc.semaphore("copy_sem"))
        out_sem = stack.enter_context(nc.semaphore("out_sem"))
        block = stack.enter_context(nc.Block())

        @block.scalar
        def _(scalar):
            scalar.dma_start(out=xt_sb[:, :], in_=xt[:, :]).then_inc(xt_sem, 16)

        @block.gpsimd
        def _(gp):
            for g in range(NG):
                if g >= B:
                    gp.wait_ge(pe_sem, g - B + 1)
                gp.dma_start(
                    out=tt[:, g % B],
                    in_=tsh[g * G : (g + 1) * G].rearrange("k p n -> p k n"),
                ).then_inc(dma_sems[g % B], 16)

        @block.tensor
        def _(te):
            te.wait_ge(xt_sem, 16)
            for g in range(NG):
                te.wait_ge(dma_sems[g % B], (g // B + 1) * 16)
                last = None
                for j in range(G):
                    k = g * G + j
                    lhsT = xt_sb[:, k * SLICES : (k + 1) * SLICES]
                    for off, nn in ((0, 512), (512, 512), (1024, 128)):
                        last = nc.tensor.matmul(
                            psum[:, off : off + nn],
                            lhsT,
                            tt[:, g % B, j, off : off + nn],
                            start=(k == 0),
                            stop=(k == KC - 1),
                        )
                last.then_inc(pe_sem, 1)

        @block.vector
        def _(v):
            v.wait_ge(pe_sem, NG)
            nc.vector.tensor_copy(o_sb[:, :], psum[:, :]).then_inc(copy_sem, 1)

        @block.sync
        def _(s):
            s.wait_ge(copy_sem, 1)
            s.dma_start(out=out[:, :], in_=o_sb[:, :]).then_inc(out_sem, 16)
            s.wait_ge(out_sem, 16)

    return nc


def _get_state():
    if "state" not in _cache:
        T = _build_T()
        t_bf = T.astype(ml_dtypes.bfloat16).reshape(KC, 128, P_TOTAL)
        shards = [
            np.ascontiguousarray(t_bf[:, :, c * PSH : (c + 1) * PSH]) for c in range(NCORES)
        ]
        _cache["state"] = (shards, _build_bass())
    return _cache["state"]


def kernel(x, encoder_input_dims=None, decoder_target_shape=None, _want_perf=False):
    from concourse.bass_utils import run_bass_kernel_spmd

    shards, nc = _get_state()
    x = np.asarray(x, dtype=np.float32)
    x_flat = x.reshape(SLICES, K)
    # lhsT chunk layout: xt_sb[p, k*96+m] = x_flat[m, k*128+p]
    xt_host = np.ascontiguousarray(
        x_flat.T.reshape(KC, 128, SLICES).transpose(1, 0, 2).reshape(128, KC * SLICES)
    ).astype(ml_dtypes.bfloat16)
    in_maps = [{"xt": xt_host, "tsh": shards[c]} for c in range(NCORES)]
    res = run_bass_kernel_spmd(
        nc, in_maps, core_ids=list(range(NCORES)), trace=_want_perf
    )
    full = np.concatenate([res.results[c]["out"] for c in range(NCORES)], axis=1)
    out = full.reshape(2, 1, 48, WIDTH, WIDTH)
    if _want_perf:
        return out, res
    return out


# revision 7
# speedup vs baseline: 1.5356x; 1.5356x over previous
"""Fused FBP (ramp-filter + backprojection + flip + resize + crop) Trainium2 kernel.

The whole reference pipeline is linear in the input sinogram, so it folds into a
single constant matrix T of shape (A*DET, W*W) = (20736, 9216):

    out[n, p] = sum_k x_flat[n, k] * T[k, p]

Angles pair up under the mirror symmetry u_{pi-theta}(x,y) = u_theta(-x,y):
T[(215-i, d)] = mirror_x(T[(i, d)]), so only the first 108 angles' rows of T are
streamed; two PSUM accumulators share each T tile and the second one is
x-mirrored and added at the end:

    out = x_A @ T_half + mirror_x(x_B @ T_half)

The kernel is one PSUM-accumulated matmul per core, with T's output-pixel axis
sharded across the 8 NeuronCores (1152 columns each). T is built on host once
(numpy) and streamed from HBM in bf16; x is replicated in bf16.
"""

import numpy as np
import ml_dtypes

N_ANGLES = 216
DET = 96
WIDTH = 96
UPSAMPLE = 1.8
PAD = 256

SLICES = 96                    # 2*1*48 sinogram slices
K = N_ANGLES * DET             # 20736 contraction length
P_TOTAL = WIDTH * WIDTH        # 9216 output pixels per slice
NCORES = 8
PSH = P_TOTAL // NCORES        # 1152 output pixels per core
KH = K // 2                    # 10368 contraction rows actually streamed
KCH = KH // 128                # 81 k-chunks of 128
GH = 3                         # k-chunks per DMA group (81 = 27 * 3)
RING = 6                       # tt ring depth

_cache = {}


def _build_T_half():
    """First-108-angles half of T[(a,d), (Y,X)]; the other half is its x-mirror."""
    # --- ramp filter as a circular-convolution matrix (filt = sino @ F) ---
    n = np.concatenate((np.arange(1, PAD // 2 + 1, 2), np.arange(PAD // 2 - 1, 0, -2)))
    f = np.zeros(PAD)
    f[0] = 0.25
    f[1::2] = -1.0 / (np.pi * n) ** 2
    full = 2.0 * np.real(np.fft.fft(f))
    ramp_bins = full[: PAD // 2 + 1].astype(np.float32).astype(np.float64)
    kern = np.fft.irfft(ramp_bins, n=PAD)
    s = np.pi / (2.0 * N_ANGLES)
    jj = np.arange(DET)[:, None]
    ii = np.arange(DET)[None, :]
    F = (s * kern[(ii - jj) % PAD]).astype(np.float32)      # (DET j, DET d)

    # --- backprojection weights as hat functions: W[a,d,p] = relu(1-|d-uc|)*inb ---
    A_HALF = N_ANGLES // 2
    angles = np.linspace(0.0, np.pi, N_ANGLES).astype(np.float32).astype(np.float64)[:A_HALF]
    grid = np.arange(WIDTH) - (WIDTH - 1) / 2.0
    ys, xs = np.meshgrid(grid, grid, indexing="ij")
    t = xs[None] * np.cos(angles)[:, None, None] + ys[None] * np.sin(angles)[:, None, None]
    u = t + (DET - 1) / 2.0                                  # (A/2, W, W)
    inb = ((u >= 0.0) & (u <= DET - 1)).astype(np.float32)
    uc = np.clip(u, 0.0, DET - 1).astype(np.float32)
    uc_flat = uc.reshape(A_HALF, P_TOTAL) * inb.reshape(A_HALF, P_TOTAL)
    inb_flat = inb.reshape(A_HALF, P_TOTAL)
    d = np.arange(DET, dtype=np.float32)
    T1 = np.empty((A_HALF, DET, P_TOTAL), dtype=np.float32)
    for a in range(A_HALF):
        Wa = np.maximum(0.0, 1.0 - np.abs(d[:, None] - uc_flat[a][None, :])) * inb_flat[a][None, :]
        T1[a] = F.T @ Wa

    # --- flip both spatial dims ---
    T1 = T1.reshape(A_HALF, DET, WIDTH, WIDTH)[:, :, ::-1, ::-1]

    # --- upsample(1.8, linear, align_corners=False) + center-crop as one matrix ---
    up = int(WIDTH * UPSAMPLE)
    crop = (up - WIDTH) // 2
    coords = (np.arange(up) + 0.5) * (WIDTH / up) - 0.5
    coords = np.clip(coords, 0.0, WIDTH - 1)
    i0 = np.floor(coords).astype(np.int64)
    i1 = np.minimum(i0 + 1, WIDTH - 1)
    w = (coords - i0).astype(np.float32)
    C = np.zeros((WIDTH, up), dtype=np.float32)
    np.add.at(C, (i0, np.arange(up)), 1.0 - w)
    np.add.at(C, (i1, np.arange(up)), w)
    C = np.ascontiguousarray(C[:, crop : crop + WIDTH])      # (y in, Y out)

    # fold C along y then x: T[(a,j),(Y,X)] = sum_{y,x} T1[a,j,y,x] C[y,Y] C[x,X]
    T2 = np.tensordot(T1, C, axes=([2], [0]))                # (A/2, DET, X, Y)
    T2 = np.tensordot(T2, C, axes=([2], [0]))                # (A/2, DET, Y, X)
    return T2.reshape(KH, P_TOTAL)


def _build_bass():
    import concourse.bass as bass
    import concourse.mybir as mybir
    from contextlib import ExitStack

    NG = KCH // GH             # 27 DMA groups

    nc = bass.Bass()
    xt = nc.declare_dram_parameter("xt", [128, 2 * KCH * SLICES], mybir.dt.bfloat16, isOutput=False)
    tsh = nc.declare_dram_parameter("tsh", [KCH, 128, PSH], mybir.dt.bfloat16, isOutput=False)
    out = nc.declare_dram_parameter("out", [SLICES, PSH], mybir.dt.float32, isOutput=True)

    with ExitStack() as stack:
        xt_sb = stack.enter_context(nc.sbuf_tensor([128, 2 * KCH * SLICES], mybir.dt.bfloat16))
        tt = stack.enter_context(nc.sbuf_tensor([128, RING, GH, PSH], mybir.dt.bfloat16))
        psumA = stack.enter_context(nc.psum_tensor([SLICES, PSH], mybir.dt.float32))
        psumB = stack.enter_context(nc.psum_tensor([SLICES, PSH], mybir.dt.float32))
        o_sb = stack.enter_context(nc.sbuf_tensor([SLICES, PSH], mybir.dt.float32))
        xt_sem = stack.enter_context(nc.semaphore("xt_sem"))
        # one DMA in flight per semaphore: SWDGE completions of concurrent DMAs
        # interleave per-engine, so a shared counter cannot order them
        dma_sems = [stack.enter_context(nc.semaphore(f"dma_sem{b}")) for b in range(RING)]
        pe_sem = stack.enter_context(nc.semaphore("pe_sem"))
        copy_sem = stack.enter_context(nc.semaphore("copy_sem"))
        out_sem = stack.enter_context(nc.semaphore("out_sem"))
        block = stack.enter_context(nc.Block())

        @block.scalar
        def _(scalar):
            scalar.dma_start(out=xt_sb[:, :], in_=xt[:, :]).then_inc(xt_sem, 16)

        @block.gpsimd
        def _(gp):
            for g in range(NG):
                if g >= RING:
                    gp.wait_ge(pe_sem, g - RING + 1)
                gp.dma_start(
                    out=tt[:, g % RING],
                    in_=tsh[g * GH : (g + 1) * GH].rearrange("k p n -> p k n"),
                ).then_inc(dma_sems[g % RING], 16)

        @block.tensor
        def _(te):
            te.wait_ge(xt_sem, 16)
            for g in range(NG):
                te.wait_ge(dma_sems[g % RING], (g // RING + 1) * 16)
                last = None
                for j in range(GH):
                    k = g * GH + j
                    lhsTA = xt_sb[:, k * SLICES : (k + 1) * SLICES]
                    lhsTB = xt_sb[:, (KCH + k) * SLICES : (KCH + k + 1) * SLICES]
                    for lhsT, psum in ((lhsTA, psumA), (lhsTB, psumB)):
                        for off, nn in ((0, 512), (512, 512), (1024, 128)):
                            last = nc.tensor.matmul(
                                psum[:, off : off + nn],
                                lhsT,
                                tt[:, g % RING, j, off : off + nn],
                                start=(k == 0),
                                stop=(k == KCH - 1),
                            )
                last.then_inc(pe_sem, 1)

        @block.vector
        def _(v):
            v.wait_ge(pe_sem, NG)
            # out = A + mirror_x(B): B viewed as (96, 12 rows, 96 x) with x reversed.
            # Two steps: DVE may read only one PSUM operand per instruction.
            psumB_rev = psumB.rearrange("p (r x) -> p r x", x=WIDTH)[:, :, ::-1]
            nc.vector.tensor_copy(o_sb[:, :], psumB_rev)
            nc.vector.tensor_add(o_sb[:, :], o_sb[:, :], psumA[:, :]).then_inc(copy_sem, 1)

        @block.sync
        def _(s):
            s.wait_ge(copy_sem, 1)
            s.dma_start(out=out[:, :], in_=o_sb[:, :]).then_inc(out_sem, 16)
            s.wait_ge(out_sem, 16)

    return nc


def _get_state():
    if "state" not in _cache:
        T = _build_T_half()
        t_bf = T.astype(ml_dtypes.bfloat16).reshape(KCH, 128, P_TOTAL)
        shards = [
            np.ascontiguousarray(t_bf[:, :, c * PSH : (c + 1) * PSH]) for c in range(NCORES)
        ]
        _cache["state"] = (shards, _build_bass())
    return _cache["state"]


def _pack_lhsT(x_cols):
    """(SLICES, KH) -> (128, KCH*SLICES) with xt[p, k*96+m] = x_cols[m, k*128+p]."""
    return x_cols.T.reshape(KCH, 128, SLICES).transpose(1, 0, 2).reshape(128, KCH * SLICES)


def kernel(x, encoder_input_dims=None, decoder_target_shape=None, _want_perf=False):
    from concourse.bass_utils import run_bass_kernel_spmd

    shards, nc = _get_state()
    x = np.asarray(x, dtype=np.float32)
    x_flat = x.reshape(SLICES, K)
    xA = x_flat[:, :KH]                                   # angles 0..107
    xB = x_flat[:, KH:][:, ::-1].reshape(SLICES, KH // DET, DET)[:, :, ::-1]
    xB = xB.reshape(SLICES, KH)                           # angle 215-i at block i
    xt_host = np.ascontiguousarray(
        np.concatenate([_pack_lhsT(xA), _pack_lhsT(xB)], axis=1)
    ).astype(ml_dtypes.bfloat16)
    in_maps = [{"xt": xt_host, "tsh": shards[c]} for c in range(NCORES)]
    res = run_bass_kernel_spmd(
        nc, in_maps, core_ids=list(range(NCORES)), trace=_want_perf
    )
    full = np.concatenate([res.results[c]["out"] for c in range(NCORES)], axis=1)
    out = full.reshape(2, 1, 48, WIDTH, WIDTH)
    if _want_perf:
        return out, res
    return out


# revision 15
# speedup vs baseline: 1.5644x; 1.0187x over previous
"""Fused FBP (ramp-filter + backprojection + flip + resize + crop) Trainium2 kernel.

The whole reference pipeline is linear in the input sinogram, so it folds into a
single constant matrix T of shape (A*DET, W*W) = (20736, 9216):

    out[n, p] = sum_k x_flat[n, k] * T[k, p]

Angles pair up under the mirror symmetry u_{pi-theta}(x,y) = u_theta(-x,y):
T[(215-i, d)] = mirror_x(T[(i, d)]), so only the first 108 angles' rows of T are
streamed; two PSUM accumulators share each T tile and the second one is
x-mirrored and added at the end:

    out = x_A @ T_half + mirror_x(x_B @ T_half)

The kernel is one PSUM-accumulated matmul per core, with T's output-pixel axis
sharded across the 8 NeuronCores (1152 columns each). T is built on host once
(numpy) and streamed from HBM in bf16; x is replicated in bf16.
"""

import numpy as np
import ml_dtypes

N_ANGLES = 216
DET = 96
WIDTH = 96
UPSAMPLE = 1.8
PAD = 256

SLICES = 96                    # 2*1*48 sinogram slices
K = N_ANGLES * DET             # 20736 contraction length
P_TOTAL = WIDTH * WIDTH        # 9216 output pixels per slice
NCORES = 8
PSH = P_TOTAL // NCORES        # 1152 output pixels per core
KH = K // 2                    # 10368 contraction rows actually streamed
KCH = KH // 128                # 81 k-chunks of 128
GH = 3                         # k-chunks per DMA group (81 = 27 * 3)
RING = 6                       # tt ring depth

_cache = {}


def _build_T_half():
    """First-108-angles half of T[(a,d), (Y,X)]; the other half is its x-mirror."""
    # --- ramp filter as a circular-convolution matrix (filt = sino @ F) ---
    n = np.concatenate((np.arange(1, PAD // 2 + 1, 2), np.arange(PAD // 2 - 1, 0, -2)))
    f = np.zeros(PAD)
    f[0] = 0.25
    f[1::2] = -1.0 / (np.pi * n) ** 2
    full = 2.0 * np.real(np.fft.fft(f))
    ramp_bins = full[: PAD // 2 + 1].astype(np.float32).astype(np.float64)
    kern = np.fft.irfft(ramp_bins, n=PAD)
    s = np.pi / (2.0 * N_ANGLES)
    jj = np.arange(DET)[:, None]
    ii = np.arange(DET)[None, :]
    F = (s * kern[(ii - jj) % PAD]).astype(np.float32)      # (DET j, DET d)

    # --- backprojection weights as hat functions: W[a,d,p] = relu(1-|d-uc|)*inb ---
    A_HALF = N_ANGLES // 2
    angles = np.linspace(0.0, np.pi, N_ANGLES).astype(np.float32).astype(np.float64)[:A_HALF]
    grid = np.arange(WIDTH) - (WIDTH - 1) / 2.0
    ys, xs = np.meshgrid(grid, grid, indexing="ij")
    t = xs[None] * np.cos(angles)[:, None, None] + ys[None] * np.sin(angles)[:, None, None]
    u = t + (DET - 1) / 2.0                                  # (A/2, W, W)
    inb = ((u >= 0.0) & (u <= DET - 1)).astype(np.float32)
    uc = np.clip(u, 0.0, DET - 1).astype(np.float32)
    uc_flat = uc.reshape(A_HALF, P_TOTAL) * inb.reshape(A_HALF, P_TOTAL)
    inb_flat = inb.reshape(A_HALF, P_TOTAL)
    d = np.arange(DET, dtype=np.float32)
    T1 = np.empty((A_HALF, DET, P_TOTAL), dtype=np.float32)
    for a in range(A_HALF):
        Wa = np.maximum(0.0, 1.0 - np.abs(d[:, None] - uc_flat[a][None, :])) * inb_flat[a][None, :]
        T1[a] = F.T @ Wa

    # --- flip both spatial dims ---
    T1 = T1.reshape(A_HALF, DET, WIDTH, WIDTH)[:, :, ::-1, ::-1]

    # --- upsample(1.8, linear, align_corners=False) + center-crop as one matrix ---
    up = int(WIDTH * UPSAMPLE)
    crop = (up - WIDTH) // 2
    coords = (np.arange(up) + 0.5) * (WIDTH / up) - 0.5
    coords = np.clip(coords, 0.0, WIDTH - 1)
    i0 = np.floor(coords).astype(np.int64)
    i1 = np.minimum(i0 + 1, WIDTH - 1)
    w = (coords - i0).astype(np.float32)
    C = np.zeros((WIDTH, up), dtype=np.float32)
    np.add.at(C, (i0, np.arange(up)), 1.0 - w)
    np.add.at(C, (i1, np.arange(up)), w)
    C = np.ascontiguousarray(C[:, crop : crop + WIDTH])      # (y in, Y out)

    # fold C along y then x: T[(a,j),(Y,X)] = sum_{y,x} T1[a,j,y,x] C[y,Y] C[x,X]
    T2 = np.tensordot(T1, C, axes=([2], [0]))                # (A/2, DET, X, Y)
    T2 = np.tensordot(T2, C, axes=([2], [0]))                # (A/2, DET, Y, X)
    return T2.reshape(KH, P_TOTAL)


def _build_bass():
    import concourse.bass as bass
    import concourse.mybir as mybir
    from contextlib import ExitStack

    NG = KCH // GH             # 27 DMA groups

    nc = bass.Bass()
    xt = nc.declare_dram_parameter("xt", [128, 2 * KCH * SLICES], mybir.dt.bfloat16, isOutput=False)
    tsh = nc.declare_dram_parameter("tsh", [KCH, 128, PSH], mybir.dt.bfloat16, isOutput=False)
    out = nc.declare_dram_parameter("out", [SLICES, PSH], mybir.dt.float32, isOutput=True)

    with ExitStack() as stack:
        xt_sb = stack.enter_context(nc.sbuf_tensor([128, 2 * KCH * SLICES], mybir.dt.bfloat16))
        tt = stack.enter_context(nc.sbuf_tensor([128, RING, GH, PSH], mybir.dt.bfloat16))
        psumA = stack.enter_context(nc.psum_tensor([SLICES, PSH], mybir.dt.float32))
        psumB = stack.enter_context(nc.psum_tensor([SLICES, PSH], mybir.dt.float32))
        o_sb = stack.enter_context(nc.sbuf_tensor([SLICES, PSH], mybir.dt.float32))
        xt_sem = stack.enter_context(nc.semaphore("xt_sem"))
        # one DMA in flight per semaphore: SWDGE completions of concurrent DMAs
        # interleave per-engine, so a shared counter cannot order them
        dma_sems = [stack.enter_context(nc.semaphore(f"dma_sem{b}")) for b in range(RING)]
        pe_sem = stack.enter_context(nc.semaphore("pe_sem"))
        copy_sem = stack.enter_context(nc.semaphore("copy_sem"))
        out_sem = stack.enter_context(nc.semaphore("out_sem"))
        block = stack.enter_context(nc.Block())

        # xt upload in pieces so PE can start before the whole 4MB lands.
        # xt layout interleaves A/B per chunk (see _pack), so consumption is
        # in order; each piece gets its own sem (in-flight DMA completions on
        # one ring interleave, so a shared counter cannot order them).
        XP = 8  # xt pieces
        piece = 2 * KCH * SLICES // XP
        assert piece * XP == 2 * KCH * SLICES
        xt_sems = [stack.enter_context(nc.semaphore(f"xt_sem{i}")) for i in range(XP)]

        @block.scalar
        def _(scalar):
            for i in range(XP):
                scalar.dma_start(
                    out=xt_sb[:, i * piece : (i + 1) * piece],
                    in_=xt[:, i * piece : (i + 1) * piece],
                ).then_inc(xt_sems[i], 16)

        @block.sync
        def _(s):
            for g in range(NG):
                if g >= RING:
                    s.wait_ge(pe_sem, g - RING + 1)
                s.dma_start(
                    out=tt[:, g % RING],
                    in_=tsh[g * GH : (g + 1) * GH].rearrange("k p n -> p k n"),
                ).then_inc(dma_sems[g % RING], 16)
            s.wait_ge(copy_sem, 2)
            s.dma_start(out=out[:, :], in_=o_sb[:, :]).then_inc(out_sem, 16)
            s.wait_ge(out_sem, 16)

        @block.tensor
        def _(te):
            for g in range(NG):
                te.wait_ge(dma_sems[g % RING], (g // RING + 1) * 16)
                # chunks consumed this group: 2*GH blocks of SLICES cols each
                hi_col = (g + 1) * GH * 2 * SLICES
                need = min(XP, (hi_col + piece - 1) // piece)
                te.wait_ge(xt_sems[need - 1], 16)
                last = None
                for j in range(GH):
                    k = g * GH + j
                    lhsTA = xt_sb[:, (2 * k) * SLICES : (2 * k + 1) * SLICES]
                    lhsTB = xt_sb[:, (2 * k + 1) * SLICES : (2 * k + 2) * SLICES]
                    for lhsT, psum in ((lhsTA, psumA), (lhsTB, psumB)):
                        for off, nn in ((0, 512), (512, 512), (1024, 128)):
                            last = nc.tensor.matmul(
                                psum[:, off : off + nn],
                                lhsT,
                                tt[:, g % RING, j, off : off + nn],
                                start=(k == 0),
                                stop=(k == KCH - 1),
                            )
                last.then_inc(pe_sem, 1)

        @block.vector
        def _(v):
            v.wait_ge(pe_sem, NG)
            # out = A + mirror_x(B): B viewed as (96, 12 rows, 96 x) with x reversed.
            # Two steps: DVE may read only one PSUM operand per instruction.
            psumB_rev = psumB.rearrange("p (r x) -> p r x", x=WIDTH)[:, :, ::-1]
            nc.vector.tensor_copy(o_sb[:, :], psumB_rev).then_inc(copy_sem, 1)
            v.wait_ge(copy_sem, 1)
            nc.vector.tensor_add(o_sb[:, :], o_sb[:, :], psumA[:, :]).then_inc(copy_sem, 1)

    return nc


def _get_state():
    if "state" not in _cache:
        T = _build_T_half()
        t_bf = T.astype(ml_dtypes.bfloat16).reshape(KCH, 128, P_TOTAL)
        shards = [
            np.ascontiguousarray(t_bf[:, :, c * PSH : (c + 1) * PSH]) for c in range(NCORES)
        ]
        _cache["state"] = (shards, _build_bass())
    return _cache["state"]


def _pack_lhsT(x_cols):
    """(SLICES, KH) -> (128, KCH, SLICES) with out[p, k, m] = x_cols[m, k*128+p]."""
    return x_cols.T.reshape(KCH, 128, SLICES).transpose(1, 0, 2)


def kernel(x, encoder_input_dims=None, decoder_target_shape=None, _want_perf=False):
    from concourse.bass_utils import run_bass_kernel_spmd

    shards, nc = _get_state()
    x = np.asarray(x, dtype=np.float32)
    x_flat = x.reshape(SLICES, K)
    xA = x_flat[:, :KH]                                   # angles 0..107
    xB = x_flat[:, KH:][:, ::-1].reshape(SLICES, KH // DET, DET)[:, :, ::-1]
    xB = xB.reshape(SLICES, KH)                           # angle 215-i at block i
    # interleave A/B per chunk: xt[:, 2k] = A_k, xt[:, 2k+1] = B_k
    xt_host = np.ascontiguousarray(
        np.stack([_pack_lhsT(xA), _pack_lhsT(xB)], axis=2).reshape(128, 2 * KCH * SLICES)
    ).astype(ml_dtypes.bfloat16)
    in_maps = [{"xt": xt_host, "tsh": shards[c]} for c in range(NCORES)]
    res = run_bass_kernel_spmd(
        nc, in_maps, core_ids=list(range(NCORES)), trace=_want_perf
    )
    full = np.concatenate([res.results[c]["out"] for c in range(NCORES)], axis=1)
    out = full.reshape(2, 1, 48, WIDTH, WIDTH)
    if _want_perf:
        return out, res
    return out
